# revision 13
# baseline (speedup 1.0000x reference)
"""Trainium2 Bass kernel for nn_ACmix_49658411876423.

Math notes (verified exact vs reference, rel err ~3e-7):

The reference's attention mask is inverted (valid key positions are set to
-FLT_MAX, zero-padded halo positions keep their logits).  Consequences:
  * Interior windows (block row/col 1..5): every key is "valid" -> all logits
    equal -> uniform softmax -> output = mean of v over the 12x12 window.
  * Boundary windows: only padded positions get softmax weight, and v there
    is exactly 0 -> attention output is exactly 0.
So the whole attention branch is:  out_att = (rate1/144) * W3 @ (12x12 window
sums of x) + rate1*b3 on interior blocks, 0 on boundary blocks (block-constant).

The conv branch is affine in x with weights that can be folded host-side:
  f_att[o] = sum_c fc_w[o,c] * (proj_c x + bias_c)   (proj in {W1,W2,W3})
  out_conv[oc] = sum_{i,ky,kx} dep_w[oc,i,ky,kx] * f_att[i, oc//4, shifted]
             = sum_{ky,kx} (A[ky,kx] @ x_shift)[oc] + bias_map[oc, y, x]
with dense A[ky,kx][oc, c] = sum_i dep_w[oc,i,ky,kx] * M_i[oc//4, c].
The device kernel is therefore a dense 3x3 conv (9 shifted 256x256 matmuls
accumulated in PSUM) plus a tiny window-sum branch, data-parallel over batch.
"""

import numpy as np

import concourse.bass as bass
import concourse.mybir as mybir
import concourse.tile as tile
from concourse import bacc
from concourse.bass_utils import run_bass_kernel_spmd

B, C, H, W = 64, 256, 28, 28
HEADS, HEAD_DIM = 4, 64
BLOCK, HALO, WIN, KCONV = 4, 4, 12, 3
NB = H // BLOCK          # 7 blocks per side
NCORES = 8
BLOC = B // NCORES       # 8 batches per core
HP, WP = H + 2, W + 2    # padded spatial for the 3x3 conv
HW = H * W               # 784
NHALF = HW // 2          # 392 (fits one PSUM bank in fp32)

F32 = mybir.dt.float32
F32R = mybir.dt.float32r

# set by test harness to enable NTFF profiling
TRACE = False
last_exec_time_ns = None

_graph_cache = None


def _build_graph():
    nc = bacc.Bacc("TRN2", target_bir_lowering=False, debug=False,
                   num_devices=NCORES)

    xp_d = nc.dram_tensor("xp", [BLOC, C, HP * WP], F32R, kind="ExternalInput").ap()
    wa_d = nc.dram_tensor("wa", [128, 9, 2, C], F32R, kind="ExternalInput").ap()
    w3_d = nc.dram_tensor("w3p", [128, 2, C], F32, kind="ExternalInput").ap()
    b3_d = nc.dram_tensor("b3p", [128, 2], F32, kind="ExternalInput").ap()
    bm_d = nc.dram_tensor("bmp", [128, 2, HW], F32, kind="ExternalInput").ap()
    out_d = nc.dram_tensor("out", [BLOC, C, HW], F32, kind="ExternalOutput").ap()

    with tile.TileContext(nc) as tc:
        with (
            tc.tile_pool(name="wconst", bufs=1) as wconst,
            tc.tile_pool(name="xpool", bufs=4) as xpool,
            tc.tile_pool(name="spool", bufs=4) as spool,
            tc.tile_pool(name="opool", bufs=3) as opool,
            tc.tile_pool(name="pconv", bufs=6, space="PSUM") as pconv,
            tc.tile_pool(name="patt", bufs=2, space="PSUM") as patt,
        ):
            # constant weights, loaded once
            wa_sb = wconst.tile([128, 9, 2, C], F32R)
            w3_sb = wconst.tile([128, 2, C], F32)
            b3_sb = wconst.tile([128, 2], F32)
            bm_sb = wconst.tile([128, 2, HW], F32)
            nc.sync.dma_start(out=wa_sb[:], in_=wa_d[:])
            nc.sync.dma_start(out=w3_sb[:], in_=w3_d[:])
            nc.sync.dma_start(out=b3_sb[:], in_=b3_d[:])
            nc.sync.dma_start(out=bm_sb[:], in_=bm_d[:])

            for b in range(BLOC):
                # ---- load x (padded) for this batch: 2 k-tiles of channels
                xp_t = []
                for kt in range(2):
                    xt = xpool.tile([128, HP * WP], F32R, tag="xp")
                    nc.sync.dma_start(out=xt[:], in_=xp_d[b, kt * 128:(kt + 1) * 128, :])
                    xp_t.append(xt)

                # ---- window-sum chain (DVE tree adds; every AP <= 2 free
                # dims for the BIR verifier): 12x12 sums at interior blocks
                r3s = []
                for kt in range(2):
                    x3 = xp_t[kt][:].rearrange("p (y x) -> p y x", x=WP)

                    def xtap(dx):
                        # [p, y=28, bx=7] strided view, column tap dx
                        base = xp_t[kt][:]
                        return bass.AP(
                            tensor=base.tensor,
                            offset=base.offset + WP + 1 + dx,
                            ap=[list(base.ap[0]), [WP, H], [BLOCK, NB]]).bitcast(F32)

                    ta = spool.tile([128, H * NB], F32, tag="ta")
                    tb = spool.tile([128, H * NB], F32, tag="tb")
                    s1 = spool.tile([128, H * NB], F32, tag="s1")
                    nc.vector.tensor_add(out=ta[:], in0=xtap(0), in1=xtap(1))
                    nc.vector.tensor_add(out=tb[:], in0=xtap(2), in1=xtap(3))
                    nc.vector.tensor_add(out=s1[:], in0=ta[:], in1=tb[:])

                    def s1tap(dy):
                        # [p, by=7, bx=7] view of s1 (y-major), row tap dy
                        base = s1[:]
                        return bass.AP(
                            tensor=base.tensor,
                            offset=base.offset + dy * NB,
                            ap=[list(base.ap[0]), [BLOCK * NB, NB], [1, NB]])

                    t2a = spool.tile([128, NB * NB], F32, tag="t2a")
                    t2b = spool.tile([128, NB * NB], F32, tag="t2b")
                    s2 = spool.tile([128, NB * NB], F32, tag="s2")
                    nc.vector.tensor_add(out=t2a[:], in0=s1tap(0), in1=s1tap(1))
                    nc.vector.tensor_add(out=t2b[:], in0=s1tap(2), in1=s1tap(3))
                    nc.vector.tensor_add(out=s2[:], in0=t2a[:], in1=t2b[:])

                    # 3x3 neighborhood of block sums -> interior 5x5 windows
                    def s2tap(j):
                        base = s2[:]
                        return bass.AP(
                            tensor=base.tensor,
                            offset=base.offset + j,
                            ap=[list(base.ap[0]), [NB, NB], [1, NB - 2]])

                    u1 = spool.tile([128, NB * (NB - 2)], F32, tag="u1")
                    u2 = spool.tile([128, NB * (NB - 2)], F32, tag="u2")
                    nc.vector.tensor_add(out=u1[:], in0=s2tap(0), in1=s2tap(1))
                    nc.vector.tensor_add(out=u2[:], in0=u1[:], in1=s2tap(2))

                    def utap(i):
                        base = u2[:]
                        return bass.AP(
                            tensor=base.tensor,
                            offset=base.offset + i * (NB - 2),
                            ap=[list(base.ap[0]), [NB - 2, NB - 2], [1, NB - 2]])

                    v1 = spool.tile([128, 25], F32, tag="v1")
                    r3 = spool.tile([128, 25], F32, tag="r3")
                    nc.vector.tensor_add(out=v1[:], in0=utap(0), in1=utap(1))
                    nc.vector.tensor_add(out=r3[:], in0=v1[:], in1=utap(2))
                    r3s.append(r3)

                for mt in range(2):
                    ms = slice(mt * 128, (mt + 1) * 128)

                    # ---- conv branch: 9 shifted matmuls, 2 k-tiles, into 2
                    # PSUM halves (rows 0..13 and 14..27)
                    pc = [pconv.tile([128, NHALF], F32, tag="pc", name=f"pc{h}",
                                     padded_shape=[128, 512]) for h in range(2)]
                    for t, (ky, kx, kt) in enumerate(
                            (ky, kx, kt)
                            for ky in range(3) for kx in range(3)
                            for kt in range(2)):
                        lhsT = wa_sb[:, 3 * ky + kx, kt, ms]
                        x3 = xp_t[kt][:].rearrange("p (y x) -> p y x", x=WP)
                        for h in range(2):
                            rhs = x3[:, ky + 14 * h: ky + 14 * h + 14,
                                     kx: kx + W]
                            nc.tensor.matmul(
                                pc[h][:], lhsT, rhs,
                                start=(t == 0), stop=(t == 17))

                    # ---- attention branch matmul: [256k -> 128m] on the 5x5
                    # interior window sums
                    pa = patt.tile([128, 25], F32, tag="pa", padded_shape=[128, 512])
                    for kt in range(2):
                        nc.tensor.matmul(
                            pa[:], w3_sb[:, kt, ms], r3s[kt][:],
                            start=(kt == 0), stop=(kt == 1))

                    # ---- combine: att grid (zeros at boundary blocks) + bias
                    attf = spool.tile([128, NB * NB], F32, tag="attf")
                    nc.vector.memset(attf[:], 0.0)
                    attg = attf[:].rearrange("p (by bx) -> p by bx", bx=NB)
                    nc.vector.tensor_scalar_add(
                        out=attg[:, 1:6, 1:6],
                        in0=pa[:].rearrange("p (a b) -> p a b", b=5),
                        scalar1=b3_sb[:, mt:mt + 1])

                    # broadcast att over 4x4 blocks in two strided-copy
                    # rounds (x then y), all APs 2 free dims, on GpSimd
                    attx = spool.tile([128, NB * W], F32, tag="attx")
                    axap = attx[:]
                    for dx in range(BLOCK):
                        nc.gpsimd.tensor_copy(
                            out=bass.AP(tensor=axap.tensor,
                                        offset=axap.offset + dx,
                                        ap=[list(axap.ap[0]), [W, NB], [BLOCK, NB]]),
                            in_=attg)
                    atty = opool.tile([128, HW], F32, tag="atty")
                    ayap = atty[:]
                    for dy in range(BLOCK):
                        nc.gpsimd.tensor_copy(
                            out=bass.AP(tensor=ayap.tensor,
                                        offset=ayap.offset + dy * W,
                                        ap=[list(ayap.ap[0]), [BLOCK * W, NB], [1, W]]),
                            in_=attx[:].rearrange("p (a b) -> p a b", b=W))

                    comb = opool.tile([128, HW], F32, tag="comb")
                    nc.vector.tensor_add(out=comb[:], in0=atty[:],
                                         in1=bm_sb[:, mt, :])

                    out_sb = opool.tile([128, HW], F32, tag="osb")
                    for h in range(2):
                        nc.vector.tensor_add(
                            out=out_sb[:, h * NHALF:(h + 1) * NHALF],
                            in0=pc[h][:],
                            in1=comb[:, h * NHALF:(h + 1) * NHALF])

                    nc.sync.dma_start(out=out_d[b, ms, :], in_=out_sb[:])

    nc.compile()
    return nc


def _host_precompute(w1, b1, w2, b2, w3, b3, fc_w, dep_w, rate1, rate2):
    """Fold all the small parameters into 9 dense conv matrices + bias map."""
    f64 = np.float64
    Wsrc = [w1.astype(f64), w2.astype(f64), w3.astype(f64)]
    bsrc = [b1.astype(f64), b2.astype(f64), b3.astype(f64)]
    fc = fc_w.astype(f64)
    dw = dep_w.astype(f64)
    r1 = float(rate1[0])
    r2 = float(rate2[0])

    M9 = np.zeros((9, HEAD_DIM, C), f64)
    B9 = np.zeros((9, HEAD_DIM), f64)
    for i in range(9):
        for c in range(12):
            h = c % 4
            M9[i] += fc[i, c] * Wsrc[c // 4][h * 64:(h + 1) * 64, :]
            B9[i] += fc[i, c] * bsrc[c // 4][h * 64:(h + 1) * 64]

    g = np.arange(C) // 4
    A = np.zeros((3, 3, C, C), f64)
    bA = np.zeros((3, 3, C), f64)
    for ky in range(3):
        for kx in range(3):
            A[ky, kx] = np.einsum('oi,ioc->oc', dw[:, :, ky, kx], M9[:, g, :])
            bA[ky, kx] = np.einsum('oi,io->o', dw[:, :, ky, kx], B9[:, g])

    # position-dependent bias: sum of bA over in-bounds taps
    ky_ok = np.ones((3, H), bool)
    ky_ok[0, 0] = False          # tap ky=0 reads row y-1
    ky_ok[2, H - 1] = False      # tap ky=2 reads row y+1
    kx_ok = np.ones((3, W), bool)
    kx_ok[0, 0] = False
    kx_ok[2, W - 1] = False
    bmap = np.zeros((C, H, W), f64)
    for ky in range(3):
        for kx in range(3):
            mask = ky_ok[ky][:, None] & kx_ok[kx][None, :]
            bmap += bA[ky, kx][:, None, None] * mask[None]
    bmap *= r2

    # pack device layouts
    wa = np.empty((128, 9, 2, C), np.float32)
    for s in range(9):
        At = (r2 * A[s // 3, s % 3]).T.astype(np.float32)   # [cin, oc]
        for kt in range(2):
            wa[:, s, kt, :] = At[kt * 128:(kt + 1) * 128, :]
    w3p = np.empty((128, 2, C), np.float32)
    w3t = ((r1 / 144.0) * Wsrc[2]).T.astype(np.float32)     # [cin, oc]
    for kt in range(2):
        w3p[:, kt, :] = w3t[kt * 128:(kt + 1) * 128, :]
    b3p = (r1 * bsrc[2]).astype(np.float32).reshape(2, 128).T.copy()  # [128, 2]
    bmp = bmap.astype(np.float32).reshape(2, 128, HW).transpose(1, 0, 2).copy()
    return wa, w3p, b3p, bmp


def kernel(x, w1, b1, w2, b2, w3, b3, fc_w, dep_w, rel_height, rel_width,
           rate1, rate2):
    global _graph_cache, last_exec_time_ns
    if _graph_cache is None:
        _graph_cache = _build_graph()
    nc = _graph_cache

    wa, w3p, b3p, bmp = _host_precompute(
        w1, b1, w2, b2, w3, b3, fc_w, dep_w, rate1, rate2)

    xpad = np.pad(np.ascontiguousarray(x, dtype=np.float32),
                  ((0, 0), (0, 0), (1, 1), (1, 1)))
    xpad = xpad.reshape(B, C, HP * WP)

    in_maps = []
    for i in range(NCORES):
        in_maps.append({
            "xp": np.ascontiguousarray(xpad[i * BLOC:(i + 1) * BLOC]),
            "wa": wa, "w3p": w3p, "b3p": b3p, "bmp": bmp,
        })

    kw = {}
    if TRACE:
        import tempfile
        kw["tmpdir"] = tempfile.mkdtemp(prefix="ktrace_", dir="/tmp")
        globals()["last_trace_dir"] = kw["tmpdir"]
    res = run_bass_kernel_spmd(nc, in_maps, core_ids=list(range(NCORES)),
                               trace=TRACE, **kw)
    last_exec_time_ns = res.exec_time_ns
    out = np.concatenate([res.results[i]["out"] for i in range(NCORES)], axis=0)
    return out.reshape(B, C, H, W)


# revision 14
# speedup vs baseline: 1.0966x; 1.0966x over previous
"""Trainium2 Bass kernel for nn_ACmix_49658411876423.

Math notes (verified exact vs reference, rel err ~3e-7):

The reference's attention mask is inverted (valid key positions are set to
-FLT_MAX, zero-padded halo positions keep their logits).  Consequences:
  * Interior windows (block row/col 1..5): every key is "valid" -> all logits
    equal -> uniform softmax -> output = mean of v over the 12x12 window.
  * Boundary windows: only padded positions get softmax weight, and v there
    is exactly 0 -> attention output is exactly 0.
So the whole attention branch is:  out_att = (rate1/144) * W3 @ (12x12 window
sums of x) + rate1*b3 on interior blocks, 0 on boundary blocks (block-constant).

The conv branch is affine in x with weights folded host-side:
  out_conv[oc] = sum_{ky,kx} (A[ky,kx] @ x_shift)[oc] + bias_map[oc, y, x]
with dense A[ky,kx][oc, c] = sum_i dep_w[oc,i,ky,kx] * M_i[oc//4, c] and
bias_map of rank <= 10 (9 in-bounds tap indicators + interior-block term).

Device kernel per core (8 batches, data-parallel over 8 cores):
  * 9 shifted 256x256 f32r matmuls accumulated in PSUM (the 3x3 conv),
    batch-paired so each weight load feeds 4 matmuls.
  * window sums of x via DVE tree adds; att^T = (wsum)^T @ W3'^T via a
    swapped-operand matmul (output [25 windows, 256 oc]).
  * one K=35 "mask" matmul per PSUM tile adds the block-broadcast attention
    values AND the rank-10 bias map straight into PSUM. No broadcast copies.
"""

import numpy as np

import concourse.bass as bass
import concourse.mybir as mybir
import concourse.tile as tile
from concourse import bacc
from concourse.bass_utils import run_bass_kernel_spmd

B, C, H, W = 64, 256, 28, 28
HEADS, HEAD_DIM = 4, 64
BLOCK, HALO, WIN, KCONV = 4, 4, 12, 3
NB = H // BLOCK          # 7 blocks per side
NI = NB - 2              # 5 interior blocks per side
NCORES = 8
BLOC = B // NCORES       # 8 batches per core
HP, WP = H + 2, W + 2    # padded spatial for the 3x3 conv
HW = H * W               # 784
NHALF = HW // 2          # 392 (fits one PSUM bank in fp32)
KM = 25 + 10             # mask-matmul contraction: 25 att rows + 10 bias rows

F32 = mybir.dt.float32
F32R = mybir.dt.float32r

# set by test harness to enable NTFF profiling
TRACE = False
last_exec_time_ns = None

_graph_cache = None


def _build_graph():
    nc = bacc.Bacc("TRN2", target_bir_lowering=False, debug=False,
                   num_devices=NCORES)

    xp_d = nc.dram_tensor("xp", [BLOC, C, HP * WP], F32R, kind="ExternalInput").ap()
    wa_d = nc.dram_tensor("wa", [128, 9, 2, C], F32R, kind="ExternalInput").ap()
    w3_d = nc.dram_tensor("w3p", [128, 2, C], F32R, kind="ExternalInput").ap()
    mk_d = nc.dram_tensor("mk", [KM, HW], F32R, kind="ExternalInput").ap()
    bl_d = nc.dram_tensor("bl", [10, C], F32R, kind="ExternalInput").ap()
    out_d = nc.dram_tensor("out", [BLOC, C, HW], F32, kind="ExternalOutput").ap()

    with tile.TileContext(nc) as tc:
        with (
            tc.tile_pool(name="wconst", bufs=1) as wconst,
            tc.tile_pool(name="xpool", bufs=8) as xpool,
            tc.tile_pool(name="spool", bufs=4) as spool,
            tc.tile_pool(name="opool", bufs=4) as opool,
            tc.tile_pool(name="pconv", bufs=6, space="PSUM") as pconv,
            tc.tile_pool(name="patt", bufs=2, space="PSUM") as patt,
        ):
            # constants, loaded once
            wa_sb = wconst.tile([128, 9, 2, C], F32R)
            w3_sb = wconst.tile([128, 2, C], F32R)
            mk_sb = wconst.tile([KM, HW], F32R)
            nc.sync.dma_start(out=wa_sb[:], in_=wa_d[:])
            nc.sync.dma_start(out=w3_sb[:], in_=w3_d[:])
            nc.sync.dma_start(out=mk_sb[:], in_=mk_d[:])
            # ping-pong combined (att + bias) lhsT tiles; bias rows 25..34
            # are written once here, att rows 0..24 are refreshed per batch
            attL = [wconst.tile([KM, C], F32R, name=f"attL{i}") for i in range(2)]
            for i in range(2):
                nc.sync.dma_start(out=attL[i][25:, :], in_=bl_d[:])

            for bp in range(BLOC // 2):
                bots = [2 * bp, 2 * bp + 1]
                xp_t = {}
                for b in bots:
                    for kt in range(2):
                        xt = xpool.tile([128, HP * WP], F32R, tag="xp",
                                        name=f"xt{b % 2}{kt}")
                        nc.sync.dma_start(
                            out=xt[:], in_=xp_d[b, kt * 128:(kt + 1) * 128, :])
                        xp_t[(b, kt)] = xt

                # ---- window-sum chain (DVE tree adds, <=2 free dims each):
                # 12x12 sums at the 5x5 interior blocks -> r3 [128, 25] f32r
                r3s = {}
                for b in bots:
                    for kt in range(2):
                        base = xp_t[(b, kt)][:]

                        def xtap(dx, base=base):
                            return bass.AP(
                                tensor=base.tensor,
                                offset=base.offset + WP + 1 + dx,
                                ap=[list(base.ap[0]), [WP, H], [BLOCK, NB]],
                            ).bitcast(F32)

                        ta = spool.tile([128, H * NB], F32, tag="ta")
                        tb = spool.tile([128, H * NB], F32, tag="tb")
                        s1 = spool.tile([128, H * NB], F32, tag="s1")
                        nc.vector.tensor_add(out=ta[:], in0=xtap(0), in1=xtap(1))
                        nc.vector.tensor_add(out=tb[:], in0=xtap(2), in1=xtap(3))
                        nc.vector.tensor_add(out=s1[:], in0=ta[:], in1=tb[:])

                        def s1tap(dy):
                            sb = s1[:]
                            return bass.AP(
                                tensor=sb.tensor, offset=sb.offset + dy * NB,
                                ap=[list(sb.ap[0]), [BLOCK * NB, NB], [1, NB]])

                        t2a = spool.tile([128, NB * NB], F32, tag="t2a")
                        t2b = spool.tile([128, NB * NB], F32, tag="t2b")
                        s2 = spool.tile([128, NB * NB], F32, tag="s2")
                        nc.vector.tensor_add(out=t2a[:], in0=s1tap(0), in1=s1tap(1))
                        nc.vector.tensor_add(out=t2b[:], in0=s1tap(2), in1=s1tap(3))
                        nc.vector.tensor_add(out=s2[:], in0=t2a[:], in1=t2b[:])

                        def s2tap(j):
                            sb = s2[:]
                            return bass.AP(
                                tensor=sb.tensor, offset=sb.offset + j,
                                ap=[list(sb.ap[0]), [NB, NB], [1, NI]])

                        u1 = spool.tile([128, NB * NI], F32, tag="u1")
                        u2 = spool.tile([128, NB * NI], F32, tag="u2")
                        nc.vector.tensor_add(out=u1[:], in0=s2tap(0), in1=s2tap(1))
                        nc.vector.tensor_add(out=u2[:], in0=u1[:], in1=s2tap(2))

                        def utap(i):
                            ub = u2[:]
                            return bass.AP(
                                tensor=ub.tensor, offset=ub.offset + i * NI,
                                ap=[list(ub.ap[0]), [NI, NI], [1, NI]])

                        v1 = spool.tile([128, 25], F32, tag="v1")
                        r3 = spool.tile([128, 25], F32R, tag="r3",
                                        name=f"r3{b % 2}{kt}")
                        nc.vector.tensor_add(out=v1[:], in0=utap(0), in1=utap(1))
                        nc.vector.tensor_add(out=r3[:], in0=v1[:].bitcast(F32R),
                                             in1=utap(2).bitcast(F32R))
                        r3s[(b, kt)] = r3

                # ---- att^T: paT[25 win, 256 oc] = wsum^T @ W3'^T, then into
                # the ping-pong lhsT tile (rows 0..24)
                for b in bots:
                    paT = patt.tile([25, C], F32, tag="paT",
                                    padded_shape=[25, 512])
                    for kt in range(2):
                        nc.tensor.matmul(paT[:], r3s[(b, kt)][:],
                                         w3_sb[:, kt, :],
                                         start=(kt == 0), stop=(kt == 1))
                    nc.vector.tensor_copy(out=attL[b % 2][0:25, :],
                                          in_=paT[:].bitcast(F32R))

                # ---- conv + mask matmuls, batch-paired for weight reuse
                for mt in range(2):
                    ms = slice(mt * 128, (mt + 1) * 128)
                    pc = {(b, h): pconv.tile([128, NHALF], F32, tag="pc",
                                             name=f"pc{b % 2}{h}",
                                             padded_shape=[128, 512])
                          for b in bots for h in range(2)}
                    for t, (ky, kx, kt) in enumerate(
                            (ky, kx, kt)
                            for ky in range(3) for kx in range(3)
                            for kt in range(2)):
                        lhsT = wa_sb[:, 3 * ky + kx, kt, ms]
                        for b in bots:
                            x3 = xp_t[(b, kt)][:].rearrange(
                                "p (y x) -> p y x", x=WP)
                            for h in range(2):
                                rhs = x3[:, ky + 14 * h: ky + 14 * h + 14,
                                         kx: kx + W]
                                nc.tensor.matmul(
                                    pc[(b, h)][:], lhsT, rhs,
                                    start=(t == 0), stop=False)
                    # att broadcast + rank-10 bias map, straight into PSUM
                    for b in bots:
                        for h in range(2):
                            nc.tensor.matmul(
                                pc[(b, h)][:], attL[b % 2][:, ms],
                                mk_sb[:, h * NHALF:(h + 1) * NHALF],
                                start=False, stop=True)

                    for b in bots:
                        out_sb = opool.tile([128, HW], F32, tag="osb",
                                            name=f"osb{b % 2}")
                        for h in range(2):
                            nc.vector.tensor_copy(
                                out=out_sb[:, h * NHALF:(h + 1) * NHALF],
                                in_=pc[(b, h)][:])
                        nc.sync.dma_start(out=out_d[b, ms, :], in_=out_sb[:])

    nc.compile()
    return nc


def _host_precompute(w1, b1, w2, b2, w3, b3, fc_w, dep_w, rate1, rate2):
    """Fold all the small parameters into 9 dense conv matrices, the
    window-attention projection, and the rank-10 bias-map factors."""
    f64 = np.float64
    Wsrc = [w1.astype(f64), w2.astype(f64), w3.astype(f64)]
    bsrc = [b1.astype(f64), b2.astype(f64), b3.astype(f64)]
    fc = fc_w.astype(f64)
    dw = dep_w.astype(f64)
    r1 = float(rate1[0])
    r2 = float(rate2[0])

    M9 = np.zeros((9, HEAD_DIM, C), f64)
    B9 = np.zeros((9, HEAD_DIM), f64)
    for i in range(9):
        for c in range(12):
            h = c % 4
            M9[i] += fc[i, c] * Wsrc[c // 4][h * 64:(h + 1) * 64, :]
            B9[i] += fc[i, c] * bsrc[c // 4][h * 64:(h + 1) * 64]

    g = np.arange(C) // 4
    A = np.zeros((9, C, C), f64)      # A[s = ky*3+kx]
    bA = np.zeros((9, C), f64)
    for ky in range(3):
        for kx in range(3):
            s = ky * 3 + kx
            A[s] = np.einsum('oi,ioc->oc', dw[:, :, ky, kx], M9[:, g, :])
            bA[s] = np.einsum('oi,io->o', dw[:, :, ky, kx], B9[:, g])

    # conv weights, rate2-folded, lhsT layout [k-part, tap, k-tile, oc]
    wa = np.empty((128, 9, 2, C), np.float32)
    for s in range(9):
        At = (r2 * A[s]).T.astype(np.float32)              # [cin, oc]
        for kt in range(2):
            wa[:, s, kt, :] = At[kt * 128:(kt + 1) * 128, :]
    # attention projection, (rate1/144)-folded, [k-part, k-tile, oc]
    w3p = np.empty((128, 2, C), np.float32)
    w3t = ((r1 / 144.0) * Wsrc[2]).T.astype(np.float32)
    for kt in range(2):
        w3p[:, kt, :] = w3t[kt * 128:(kt + 1) * 128, :]

    # mask rhs [35, 784]: rows 0..24 one-hot interior-block indicator,
    # rows 25..33 in-bounds tap indicators, row 34 interior indicator
    yy, xx = np.meshgrid(np.arange(H), np.arange(W), indexing='ij')
    by, bx = yy // BLOCK, xx // BLOCK
    mk = np.zeros((KM, H, W), np.float32)
    for wy in range(NI):
        for wx in range(NI):
            mk[wy * NI + wx] = ((by == wy + 1) & (bx == wx + 1))
    for ky in range(3):
        for kx in range(3):
            ok_y = np.ones(H, bool)
            if ky == 0:
                ok_y[0] = False
            if ky == 2:
                ok_y[H - 1] = False
            ok_x = np.ones(W, bool)
            if kx == 0:
                ok_x[0] = False
            if kx == 2:
                ok_x[W - 1] = False
            mk[25 + ky * 3 + kx] = ok_y[:, None] & ok_x[None, :]
    interior = (by >= 1) & (by <= NB - 2) & (bx >= 1) & (bx <= NB - 2)
    mk[34] = interior
    mk = mk.reshape(KM, HW)

    # bias lhsT rows [10, 256]: rate2-folded tap biases + att bias
    bl = np.empty((10, C), np.float32)
    for s in range(9):
        bl[s] = (r2 * bA[s]).astype(np.float32)
    bl[9] = (r1 * bsrc[2]).astype(np.float32)

    return wa, w3p, mk, bl


def kernel(x, w1, b1, w2, b2, w3, b3, fc_w, dep_w, rel_height, rel_width,
           rate1, rate2):
    global _graph_cache, last_exec_time_ns
    if _graph_cache is None:
        _graph_cache = _build_graph()
    nc = _graph_cache

    wa, w3p, mk, bl = _host_precompute(
        w1, b1, w2, b2, w3, b3, fc_w, dep_w, rate1, rate2)

    xpad = np.pad(np.ascontiguousarray(x, dtype=np.float32),
                  ((0, 0), (0, 0), (1, 1), (1, 1)))
    xpad = xpad.reshape(B, C, HP * WP)

    in_maps = []
    for i in range(NCORES):
        in_maps.append({
            "xp": np.ascontiguousarray(xpad[i * BLOC:(i + 1) * BLOC]),
            "wa": wa, "w3p": w3p, "mk": mk, "bl": bl,
        })

    kw = {}
    if TRACE:
        import tempfile
        kw["tmpdir"] = tempfile.mkdtemp(prefix="ktrace_", dir="/tmp")
        globals()["last_trace_dir"] = kw["tmpdir"]
    res = run_bass_kernel_spmd(nc, in_maps, core_ids=list(range(NCORES)),
                               trace=TRACE, **kw)
    last_exec_time_ns = res.exec_time_ns
    out = np.concatenate([res.results[i]["out"] for i in range(NCORES)], axis=0)
    return out.reshape(B, C, H, W)


# revision 15
# speedup vs baseline: 1.1848x; 1.0804x over previous
"""Trainium2 Bass kernel for nn_ACmix_49658411876423.

Math notes (verified exact vs reference, rel err ~3e-7):

The reference's attention mask is inverted (valid key positions are set to
-FLT_MAX, zero-padded halo positions keep their logits).  Consequences:
  * Interior windows (block row/col 1..5): every key is "valid" -> all logits
    equal -> uniform softmax -> output = mean of v over the 12x12 window.
  * Boundary windows: only padded positions get softmax weight, and v there
    is exactly 0 -> attention output is exactly 0.
So the whole attention branch is:  out_att = (rate1/144) * W3 @ (12x12 window
sums of x) + rate1*b3 on interior blocks, 0 on boundary blocks (block-constant).

The conv branch is affine in x with weights folded host-side:
  out_conv[oc] = sum_{ky,kx} (A[ky,kx] @ x_shift)[oc] + bias_map[oc, y, x]
with dense A[ky,kx][oc, c] = sum_i dep_w[oc,i,ky,kx] * M_i[oc//4, c] and
bias_map of rank <= 10 (9 in-bounds tap indicators + interior-block term).

Device kernel per core (8 batches, data-parallel over 8 cores):
  * 9 shifted 256x256 f32r matmuls accumulated in PSUM (the 3x3 conv),
    batch-paired so each weight load feeds 4 matmuls.
  * window sums of x via DVE tree adds; att^T = (wsum)^T @ W3'^T via a
    swapped-operand matmul (output [25 windows, 256 oc]).
  * one K=35 "mask" matmul per PSUM tile adds the block-broadcast attention
    values AND the rank-10 bias map straight into PSUM. No broadcast copies.
"""

import numpy as np

import concourse.bass as bass
import concourse.mybir as mybir
import concourse.tile as tile
from concourse import bacc
from concourse.bass_utils import run_bass_kernel_spmd

B, C, H, W = 64, 256, 28, 28
HEADS, HEAD_DIM = 4, 64
BLOCK, HALO, WIN, KCONV = 4, 4, 12, 3
NB = H // BLOCK          # 7 blocks per side
NI = NB - 2              # 5 interior blocks per side
NCORES = 8
BLOC = B // NCORES       # 8 batches per core
HP, WP = H + 2, W + 2    # padded spatial for the 3x3 conv
HW = H * W               # 784
NHALF = HW // 2          # 392 (fits one PSUM bank in fp32)
KM = 25 + 10             # mask-matmul contraction: 25 att rows + 10 bias rows

F32 = mybir.dt.float32
F32R = mybir.dt.float32r
BF16 = mybir.dt.bfloat16

# conv-matmul input dtype: "bf16" (fast: 1 cyc/row + fast weight load) or
# "f32r" (1.44 cyc/row, ~30x lower quantization error)
CONV_DTYPE = "bf16"
CDT = BF16 if CONV_DTYPE == "bf16" else F32R

# set by test harness to enable NTFF profiling
TRACE = False
last_exec_time_ns = None

_graph_cache = None


def _build_graph():
    nc = bacc.Bacc("TRN2", target_bir_lowering=False, debug=False,
                   num_devices=NCORES)

    xp_d = nc.dram_tensor("xp", [BLOC, C, HP * WP], CDT, kind="ExternalInput").ap()
    wa_d = nc.dram_tensor("wa", [128, 9, 2, C], CDT, kind="ExternalInput").ap()
    w3_d = nc.dram_tensor("w3p", [128, 2, C], F32R, kind="ExternalInput").ap()
    mk_d = nc.dram_tensor("mk", [KM, HW], F32R, kind="ExternalInput").ap()
    bl_d = nc.dram_tensor("bl", [10, C], F32R, kind="ExternalInput").ap()
    out_d = nc.dram_tensor("out", [BLOC, C, HW], F32, kind="ExternalOutput").ap()

    with tile.TileContext(nc) as tc:
        with (
            tc.tile_pool(name="wconst", bufs=1) as wconst,
            tc.tile_pool(name="xpool", bufs=8) as xpool,
            tc.tile_pool(name="spool", bufs=4) as spool,
            tc.tile_pool(name="opool", bufs=4) as opool,
            tc.tile_pool(name="pconv", bufs=6, space="PSUM") as pconv,
            tc.tile_pool(name="patt", bufs=2, space="PSUM") as patt,
        ):
            # constants, loaded once
            wa_sb = wconst.tile([128, 9, 2, C], CDT)
            w3_sb = wconst.tile([128, 2, C], F32R)
            mk_sb = wconst.tile([KM, HW], F32R)
            for s in range(9):
                nc.sync.dma_start(out=wa_sb[:, s, :, :], in_=wa_d[:, s, :, :])
            nc.sync.dma_start(out=w3_sb[:], in_=w3_d[:])
            nc.sync.dma_start(out=mk_sb[:], in_=mk_d[:])
            # ping-pong combined (att + bias) lhsT tiles; bias rows 25..34
            # are written once here, att rows 0..24 are refreshed per batch
            attL = [wconst.tile([KM, C], F32R, name=f"attL{i}") for i in range(2)]
            for i in range(2):
                nc.sync.dma_start(out=attL[i][25:, :], in_=bl_d[:])

            for bp in range(BLOC // 2):
                bots = [2 * bp, 2 * bp + 1]
                xp_t = {}
                for b in bots:
                    for kt in range(2):
                        xt = xpool.tile([128, HP * WP], CDT, tag="xp",
                                        name=f"xt{b % 2}{kt}")
                        nc.sync.dma_start(
                            out=xt[:], in_=xp_d[b, kt * 128:(kt + 1) * 128, :])
                        xp_t[(b, kt)] = xt

                # ---- window-sum chain (DVE tree adds, <=2 free dims each):
                # 12x12 sums at the 5x5 interior blocks -> r3 [128, 25] f32r
                r3s = {}
                for b in bots:
                    for kt in range(2):
                        base = xp_t[(b, kt)][:]

                        def xtap(dx, base=base):
                            ap = bass.AP(
                                tensor=base.tensor,
                                offset=base.offset + WP + 1 + dx,
                                ap=[list(base.ap[0]), [WP, H], [BLOCK, NB]])
                            return ap if CDT == BF16 else ap.bitcast(F32)

                        ta = spool.tile([128, H * NB], F32, tag="ta")
                        tb = spool.tile([128, H * NB], F32, tag="tb")
                        s1 = spool.tile([128, H * NB], F32, tag="s1")
                        nc.vector.tensor_add(out=ta[:], in0=xtap(0), in1=xtap(1))
                        nc.vector.tensor_add(out=tb[:], in0=xtap(2), in1=xtap(3))
                        nc.vector.tensor_add(out=s1[:], in0=ta[:], in1=tb[:])

                        def s1tap(dy):
                            sb = s1[:]
                            return bass.AP(
                                tensor=sb.tensor, offset=sb.offset + dy * NB,
                                ap=[list(sb.ap[0]), [BLOCK * NB, NB], [1, NB]])

                        t2a = spool.tile([128, NB * NB], F32, tag="t2a")
                        t2b = spool.tile([128, NB * NB], F32, tag="t2b")
                        s2 = spool.tile([128, NB * NB], F32, tag="s2")
                        nc.vector.tensor_add(out=t2a[:], in0=s1tap(0), in1=s1tap(1))
                        nc.vector.tensor_add(out=t2b[:], in0=s1tap(2), in1=s1tap(3))
                        nc.vector.tensor_add(out=s2[:], in0=t2a[:], in1=t2b[:])

                        def s2tap(j):
                            sb = s2[:]
                            return bass.AP(
                                tensor=sb.tensor, offset=sb.offset + j,
                                ap=[list(sb.ap[0]), [NB, NB], [1, NI]])

                        u1 = spool.tile([128, NB * NI], F32, tag="u1")
                        u2 = spool.tile([128, NB * NI], F32, tag="u2")
                        nc.vector.tensor_add(out=u1[:], in0=s2tap(0), in1=s2tap(1))
                        nc.vector.tensor_add(out=u2[:], in0=u1[:], in1=s2tap(2))

                        def utap(i):
                            ub = u2[:]
                            return bass.AP(
                                tensor=ub.tensor, offset=ub.offset + i * NI,
                                ap=[list(ub.ap[0]), [NI, NI], [1, NI]])

                        v1 = spool.tile([128, 25], F32, tag="v1")
                        r3 = spool.tile([128, 25], F32R, tag="r3",
                                        name=f"r3{b % 2}{kt}")
                        nc.vector.tensor_add(out=v1[:], in0=utap(0), in1=utap(1))
                        nc.vector.tensor_add(out=r3[:], in0=v1[:].bitcast(F32R),
                                             in1=utap(2).bitcast(F32R))
                        r3s[(b, kt)] = r3

                # ---- att^T: paT[25 win, 256 oc] = wsum^T @ W3'^T, then into
                # the ping-pong lhsT tile (rows 0..24)
                for b in bots:
                    paT = patt.tile([25, C], F32, tag="paT",
                                    padded_shape=[25, 512])
                    for kt in range(2):
                        nc.tensor.matmul(paT[:], r3s[(b, kt)][:],
                                         w3_sb[:, kt, :],
                                         start=(kt == 0), stop=(kt == 1))
                    nc.vector.tensor_copy(out=attL[b % 2][0:25, :],
                                          in_=paT[:].bitcast(F32R))

                # ---- conv + mask matmuls, batch-paired for weight reuse
                for mt in range(2):
                    ms = slice(mt * 128, (mt + 1) * 128)
                    pc = {(b, h): pconv.tile([128, NHALF], F32, tag="pc",
                                             name=f"pc{b % 2}{h}",
                                             padded_shape=[128, 512])
                          for b in bots for h in range(2)}
                    for t, (ky, kx, kt) in enumerate(
                            (ky, kx, kt)
                            for ky in range(3) for kx in range(3)
                            for kt in range(2)):
                        lhsT = wa_sb[:, 3 * ky + kx, kt, ms]
                        for b in bots:
                            x3 = xp_t[(b, kt)][:].rearrange(
                                "p (y x) -> p y x", x=WP)
                            for h in range(2):
                                rhs = x3[:, ky + 14 * h: ky + 14 * h + 14,
                                         kx: kx + W]
                                nc.tensor.matmul(
                                    pc[(b, h)][:], lhsT, rhs,
                                    start=(t == 0), stop=False)
                    # att broadcast + rank-10 bias map, straight into PSUM
                    for b in bots:
                        for h in range(2):
                            nc.tensor.matmul(
                                pc[(b, h)][:], attL[b % 2][:, ms],
                                mk_sb[:, h * NHALF:(h + 1) * NHALF],
                                start=False, stop=True)

                    for b in bots:
                        out_sb = opool.tile([128, HW], F32, tag="osb",
                                            name=f"osb{b % 2}")
                        for h in range(2):
                            nc.vector.tensor_copy(
                                out=out_sb[:, h * NHALF:(h + 1) * NHALF],
                                in_=pc[(b, h)][:])
                        nc.sync.dma_start(out=out_d[b, ms, :], in_=out_sb[:])

    nc.compile()
    return nc


def _host_precompute(w1, b1, w2, b2, w3, b3, fc_w, dep_w, rate1, rate2):
    """Fold all the small parameters into 9 dense conv matrices, the
    window-attention projection, and the rank-10 bias-map factors."""
    f64 = np.float64
    Wsrc = [w1.astype(f64), w2.astype(f64), w3.astype(f64)]
    bsrc = [b1.astype(f64), b2.astype(f64), b3.astype(f64)]
    fc = fc_w.astype(f64)
    dw = dep_w.astype(f64)
    r1 = float(rate1[0])
    r2 = float(rate2[0])

    M9 = np.zeros((9, HEAD_DIM, C), f64)
    B9 = np.zeros((9, HEAD_DIM), f64)
    for i in range(9):
        for c in range(12):
            h = c % 4
            M9[i] += fc[i, c] * Wsrc[c // 4][h * 64:(h + 1) * 64, :]
            B9[i] += fc[i, c] * bsrc[c // 4][h * 64:(h + 1) * 64]

    g = np.arange(C) // 4
    A = np.zeros((9, C, C), f64)      # A[s = ky*3+kx]
    bA = np.zeros((9, C), f64)
    for ky in range(3):
        for kx in range(3):
            s = ky * 3 + kx
            A[s] = np.einsum('oi,ioc->oc', dw[:, :, ky, kx], M9[:, g, :])
            bA[s] = np.einsum('oi,io->o', dw[:, :, ky, kx], B9[:, g])

    # conv weights, rate2-folded, lhsT layout [k-part, tap, k-tile, oc]
    wa = np.empty((128, 9, 2, C), np.float32)
    for s in range(9):
        At = (r2 * A[s]).T.astype(np.float32)              # [cin, oc]
        for kt in range(2):
            wa[:, s, kt, :] = At[kt * 128:(kt + 1) * 128, :]
    # attention projection, (rate1/144)-folded, [k-part, k-tile, oc]
    w3p = np.empty((128, 2, C), np.float32)
    w3t = ((r1 / 144.0) * Wsrc[2]).T.astype(np.float32)
    for kt in range(2):
        w3p[:, kt, :] = w3t[kt * 128:(kt + 1) * 128, :]

    # mask rhs [35, 784]: rows 0..24 one-hot interior-block indicator,
    # rows 25..33 in-bounds tap indicators, row 34 interior indicator
    yy, xx = np.meshgrid(np.arange(H), np.arange(W), indexing='ij')
    by, bx = yy // BLOCK, xx // BLOCK
    mk = np.zeros((KM, H, W), np.float32)
    for wy in range(NI):
        for wx in range(NI):
            mk[wy * NI + wx] = ((by == wy + 1) & (bx == wx + 1))
    for ky in range(3):
        for kx in range(3):
            ok_y = np.ones(H, bool)
            if ky == 0:
                ok_y[0] = False
            if ky == 2:
                ok_y[H - 1] = False
            ok_x = np.ones(W, bool)
            if kx == 0:
                ok_x[0] = False
            if kx == 2:
                ok_x[W - 1] = False
            mk[25 + ky * 3 + kx] = ok_y[:, None] & ok_x[None, :]
    interior = (by >= 1) & (by <= NB - 2) & (bx >= 1) & (bx <= NB - 2)
    mk[34] = interior
    mk = mk.reshape(KM, HW)

    # bias lhsT rows [10, 256]: rate2-folded tap biases + att bias
    bl = np.empty((10, C), np.float32)
    for s in range(9):
        bl[s] = (r2 * bA[s]).astype(np.float32)
    bl[9] = (r1 * bsrc[2]).astype(np.float32)

    return wa, w3p, mk, bl


def kernel(x, w1, b1, w2, b2, w3, b3, fc_w, dep_w, rel_height, rel_width,
           rate1, rate2):
    global _graph_cache, last_exec_time_ns
    if _graph_cache is None:
        _graph_cache = _build_graph()
    nc = _graph_cache

    wa, w3p, mk, bl = _host_precompute(
        w1, b1, w2, b2, w3, b3, fc_w, dep_w, rate1, rate2)

    import ml_dtypes
    np_cdt = ml_dtypes.bfloat16 if CONV_DTYPE == "bf16" else np.float32
    wa = wa.astype(np_cdt)
    xpad = np.pad(np.ascontiguousarray(x, dtype=np.float32),
                  ((0, 0), (0, 0), (1, 1), (1, 1)))
    xpad = xpad.reshape(B, C, HP * WP).astype(np_cdt)

    in_maps = []
    for i in range(NCORES):
        in_maps.append({
            "xp": np.ascontiguousarray(xpad[i * BLOC:(i + 1) * BLOC]),
            "wa": wa, "w3p": w3p, "mk": mk, "bl": bl,
        })

    kw = {}
    if TRACE:
        import tempfile
        kw["tmpdir"] = tempfile.mkdtemp(prefix="ktrace_", dir="/tmp")
        globals()["last_trace_dir"] = kw["tmpdir"]
    res = run_bass_kernel_spmd(nc, in_maps, core_ids=list(range(NCORES)),
                               trace=TRACE, **kw)
    last_exec_time_ns = res.exec_time_ns
    out = np.concatenate([res.results[i]["out"] for i in range(NCORES)], axis=0)
    return out.reshape(B, C, H, W)


# revision 19
# speedup vs baseline: 1.4041x; 1.1851x over previous
"""Trainium2 Bass kernel for nn_ACmix_49658411876423.

Math notes (verified exact vs reference, rel err ~3e-7):

The reference's attention mask is inverted (valid key positions are set to
-FLT_MAX, zero-padded halo positions keep their logits).  Consequences:
  * Interior windows (block row/col 1..5): every key is "valid" -> all logits
    equal -> uniform softmax -> output = mean of v over the 12x12 window.
  * Boundary windows: only padded positions get softmax weight, and v there
    is exactly 0 -> attention output is exactly 0.
So the whole attention branch is:  out_att = (rate1/144) * W3 @ (12x12 window
sums of x) + rate1*b3 on interior blocks, 0 on boundary blocks (block-constant).

The conv branch is affine in x with weights folded host-side:
  out_conv[oc] = sum_{ky,kx} (A[ky,kx] @ x_shift)[oc] + bias_map[oc, y, x]
with dense A[ky,kx][oc, c] = sum_i dep_w[oc,i,ky,kx] * M_i[oc//4, c] and
bias_map of rank <= 10 (9 in-bounds tap indicators + interior-block term).

Device kernel per core (8 batches, data-parallel over 8 cores):
  * 9 shifted 256x256 f32r matmuls accumulated in PSUM (the 3x3 conv),
    batch-paired so each weight load feeds 4 matmuls.
  * window sums of x via DVE tree adds; att^T = (wsum)^T @ W3'^T via a
    swapped-operand matmul (output [25 windows, 256 oc]).
  * one K=35 "mask" matmul per PSUM tile adds the block-broadcast attention
    values AND the rank-10 bias map straight into PSUM. No broadcast copies.
"""

import numpy as np

import concourse.bass as bass
import concourse.mybir as mybir
import concourse.tile as tile
from concourse import bacc
from concourse.bass_utils import run_bass_kernel_spmd

B, C, H, W = 64, 256, 28, 28
HEADS, HEAD_DIM = 4, 64
BLOCK, HALO, WIN, KCONV = 4, 4, 12, 3
NB = H // BLOCK          # 7 blocks per side
NI = NB - 2              # 5 interior blocks per side
NCORES = 8
BLOC = B // NCORES       # 8 batches per core
HP, WP = H + 2, W + 2    # padded spatial for the 3x3 conv
HW = H * W               # 784
NHALF = HW // 2          # 392 (fits one PSUM bank in fp32)
KM = 25 + 10             # mask-matmul contraction: 25 att rows + 10 bias rows

F32 = mybir.dt.float32
F32R = mybir.dt.float32r
BF16 = mybir.dt.bfloat16

# conv-matmul input dtype: "bf16" (fast: 1 cyc/row + fast weight load) or
# "f32r" (1.44 cyc/row, ~30x lower quantization error)
CONV_DTYPE = "bf16"
CDT = BF16 if CONV_DTYPE == "bf16" else F32R

# set by test harness to enable NTFF profiling
TRACE = False
last_exec_time_ns = None

_graph_cache = None


def _build_graph():
    nc = bacc.Bacc("TRN2", target_bir_lowering=False, debug=False,
                   num_devices=NCORES)

    xp_d = nc.dram_tensor("xp", [BLOC, C, HP * WP], CDT, kind="ExternalInput").ap()
    wa_d = nc.dram_tensor("wa", [128, 9, 2, C], CDT, kind="ExternalInput").ap()
    w3_d = nc.dram_tensor("w3p", [128, 2, C], BF16, kind="ExternalInput").ap()
    mk_d = nc.dram_tensor("mk", [KM, H * WP], BF16, kind="ExternalInput").ap()
    bl_d = nc.dram_tensor("bl", [10, C], BF16, kind="ExternalInput").ap()
    out_d = nc.dram_tensor("out", [BLOC, C, HW], F32, kind="ExternalOutput").ap()

    with tile.TileContext(nc) as tc:
        with (
            tc.tile_pool(name="wconst", bufs=1) as wconst,
            tc.tile_pool(name="xpool", bufs=8) as xpool,
            tc.tile_pool(name="spool", bufs=4) as spool,
            tc.tile_pool(name="opool", bufs=4) as opool,
            tc.tile_pool(name="pconv", bufs=6, space="PSUM") as pconv,
            tc.tile_pool(name="patt", bufs=2, space="PSUM") as patt,
        ):
            # constants, loaded once
            wa_sb = wconst.tile([128, 9, 2, C], CDT)
            w3_sb = wconst.tile([128, 2, C], BF16)
            mk_sb = wconst.tile([KM, H * WP], BF16)
            for s in range(9):
                nc.sync.dma_start(out=wa_sb[:, s, :, :], in_=wa_d[:, s, :, :])
            nc.sync.dma_start(out=w3_sb[:], in_=w3_d[:])
            nc.sync.dma_start(out=mk_sb[:], in_=mk_d[:])
            # ping-pong combined (att + bias) lhsT tiles; bias rows 25..34
            # are written once here, att rows 0..24 are refreshed per batch
            attL = [wconst.tile([KM, C], BF16, name=f"attL{i}") for i in range(2)]
            for i in range(2):
                nc.sync.dma_start(out=attL[i][25:, :], in_=bl_d[:])

            for bp in range(BLOC // 2):
                bots = [2 * bp, 2 * bp + 1]
                xp_t = {}
                for b in bots:
                    for kt in range(2):
                        xt = xpool.tile([128, HP * WP + 4], CDT, tag="xp",
                                        name=f"xt{b % 2}{kt}")
                        nc.sync.dma_start(
                            out=xt[:, :HP * WP],
                            in_=xp_d[b, kt * 128:(kt + 1) * 128, :])
                        nc.vector.memset(xt[:, HP * WP:], 0.0)
                        xp_t[(b, kt)] = xt

                # ---- window-sum chain (DVE tree adds, <=2 free dims each):
                # 12x12 sums at the 5x5 interior blocks -> r3 [128, 25] f32r
                r3s = {}
                for b in bots:
                    for kt in range(2):
                        base = xp_t[(b, kt)][:]

                        def xtap(dx, base=base):
                            ap = bass.AP(
                                tensor=base.tensor,
                                offset=base.offset + WP + 1 + dx,
                                ap=[list(base.ap[0]), [WP, H], [BLOCK, NB]])
                            return ap if CDT == BF16 else ap.bitcast(F32)

                        ta = spool.tile([128, H * NB], F32, tag="ta")
                        tb = spool.tile([128, H * NB], F32, tag="tb")
                        s1 = spool.tile([128, H * NB], F32, tag="s1")
                        nc.vector.tensor_add(out=ta[:], in0=xtap(0), in1=xtap(1))
                        nc.vector.tensor_add(out=tb[:], in0=xtap(2), in1=xtap(3))
                        nc.vector.tensor_add(out=s1[:], in0=ta[:], in1=tb[:])

                        def s1tap(dy):
                            sb = s1[:]
                            return bass.AP(
                                tensor=sb.tensor, offset=sb.offset + dy * NB,
                                ap=[list(sb.ap[0]), [BLOCK * NB, NB], [1, NB]])

                        t2a = spool.tile([128, NB * NB], F32, tag="t2a")
                        t2b = spool.tile([128, NB * NB], F32, tag="t2b")
                        s2 = spool.tile([128, NB * NB], F32, tag="s2")
                        nc.vector.tensor_add(out=t2a[:], in0=s1tap(0), in1=s1tap(1))
                        nc.vector.tensor_add(out=t2b[:], in0=s1tap(2), in1=s1tap(3))
                        nc.vector.tensor_add(out=s2[:], in0=t2a[:], in1=t2b[:])

                        def s2tap(j):
                            sb = s2[:]
                            return bass.AP(
                                tensor=sb.tensor, offset=sb.offset + j,
                                ap=[list(sb.ap[0]), [NB, NB], [1, NI]])

                        u1 = spool.tile([128, NB * NI], F32, tag="u1")
                        u2 = spool.tile([128, NB * NI], F32, tag="u2")
                        nc.vector.tensor_add(out=u1[:], in0=s2tap(0), in1=s2tap(1))
                        nc.vector.tensor_add(out=u2[:], in0=u1[:], in1=s2tap(2))

                        def utap(i):
                            ub = u2[:]
                            return bass.AP(
                                tensor=ub.tensor, offset=ub.offset + i * NI,
                                ap=[list(ub.ap[0]), [NI, NI], [1, NI]])

                        v1 = spool.tile([128, 25], F32, tag="v1")
                        r3 = spool.tile([128, 25], BF16, tag="r3",
                                        name=f"r3{b % 2}{kt}")
                        nc.vector.tensor_add(out=v1[:], in0=utap(0), in1=utap(1))
                        nc.vector.tensor_add(out=r3[:], in0=v1[:], in1=utap(2))
                        r3s[(b, kt)] = r3

                # ---- att^T: paT[25 win, 256 oc] = wsum^T @ W3'^T, then into
                # the ping-pong lhsT tile (rows 0..24)
                for b in bots:
                    paT = patt.tile([25, C], F32, tag="paT",
                                    padded_shape=[25, 512])
                    for kt in range(2):
                        nc.tensor.matmul(paT[:], r3s[(b, kt)][:],
                                         w3_sb[:, kt, :],
                                         start=(kt == 0), stop=(kt == 1))
                    nc.vector.tensor_copy(out=attL[b % 2][0:25, :],
                                          in_=paT[:])

                # ---- conv + mask matmuls, batch-paired for weight reuse
                for mt in range(2):
                    ms = slice(mt * 128, (mt + 1) * 128)
                    pc = {(b, h): pconv.tile([128, NHALF], F32, tag="pc",
                                             name=f"pc{b % 2}{h}",
                                             padded_shape=[128, 512])
                          for b in bots for h in range(2)}
                    for t, (ky, kx, kt) in enumerate(
                            (ky, kx, kt)
                            for ky in range(3) for kx in range(3)
                            for kt in range(2)):
                        lhsT = wa_sb[:, 3 * ky + kx, kt, ms]
                        for b in bots:
                            x3 = xp_t[(b, kt)][:, :HP * WP].rearrange(
                                "p (y x) -> p y x", x=WP)
                            for h in range(2):
                                rhs = x3[:, ky + 14 * h: ky + 14 * h + 14,
                                         kx: kx + W]
                                nc.tensor.matmul(
                                    pc[(b, h)][:], lhsT, rhs,
                                    start=(t == 0), stop=False)
                    # att broadcast + rank-10 bias map, straight into PSUM
                    for b in bots:
                        for h in range(2):
                            mkap = mk_sb[:]
                            nc.tensor.matmul(
                                pc[(b, h)][:], attL[b % 2][:, ms],
                                bass.AP(tensor=mkap.tensor,
                                        offset=mkap.offset + h * 14 * WP,
                                        ap=[list(mkap.ap[0]), [WP, 14], [1, W]]),
                                start=False, stop=True)

                    for b in bots:
                        out_sb = opool.tile([128, HW], F32, tag="osb",
                                            name=f"osb{b % 2}")
                        for h in range(2):
                            nc.vector.tensor_copy(
                                out=out_sb[:, h * NHALF:(h + 1) * NHALF],
                                in_=pc[(b, h)][:])
                        nc.sync.dma_start(out=out_d[b, ms, :], in_=out_sb[:])

    nc.compile()
    return nc


def _host_precompute(w1, b1, w2, b2, w3, b3, fc_w, dep_w, rate1, rate2):
    """Fold all the small parameters into 9 dense conv matrices, the
    window-attention projection, and the rank-10 bias-map factors."""
    f64 = np.float64
    Wsrc = [w1.astype(f64), w2.astype(f64), w3.astype(f64)]
    bsrc = [b1.astype(f64), b2.astype(f64), b3.astype(f64)]
    fc = fc_w.astype(f64)
    dw = dep_w.astype(f64)
    r1 = float(rate1[0])
    r2 = float(rate2[0])

    M9 = np.zeros((9, HEAD_DIM, C), f64)
    B9 = np.zeros((9, HEAD_DIM), f64)
    for i in range(9):
        for c in range(12):
            h = c % 4
            M9[i] += fc[i, c] * Wsrc[c // 4][h * 64:(h + 1) * 64, :]
            B9[i] += fc[i, c] * bsrc[c // 4][h * 64:(h + 1) * 64]

    g = np.arange(C) // 4
    A = np.zeros((9, C, C), f64)      # A[s = ky*3+kx]
    bA = np.zeros((9, C), f64)
    for ky in range(3):
        for kx in range(3):
            s = ky * 3 + kx
            A[s] = np.einsum('oi,ioc->oc', dw[:, :, ky, kx], M9[:, g, :])
            bA[s] = np.einsum('oi,io->o', dw[:, :, ky, kx], B9[:, g])

    # conv weights, rate2-folded, lhsT layout [k-part, tap, k-tile, oc]
    wa = np.empty((128, 9, 2, C), np.float32)
    for s in range(9):
        At = (r2 * A[s]).T.astype(np.float32)              # [cin, oc]
        for kt in range(2):
            wa[:, s, kt, :] = At[kt * 128:(kt + 1) * 128, :]
    # attention projection, (rate1/144)-folded, [k-part, k-tile, oc]
    w3p = np.empty((128, 2, C), np.float32)
    w3t = ((r1 / 144.0) * Wsrc[2]).T.astype(np.float32)
    for kt in range(2):
        w3p[:, kt, :] = w3t[kt * 128:(kt + 1) * 128, :]

    # mask rhs [35, 784]: rows 0..24 one-hot interior-block indicator,
    # rows 25..33 in-bounds tap indicators, row 34 interior indicator
    yy, xx = np.meshgrid(np.arange(H), np.arange(W), indexing='ij')
    by, bx = yy // BLOCK, xx // BLOCK
    mk = np.zeros((KM, H, W), np.float32)
    for wy in range(NI):
        for wx in range(NI):
            mk[wy * NI + wx] = ((by == wy + 1) & (bx == wx + 1))
    for ky in range(3):
        for kx in range(3):
            ok_y = np.ones(H, bool)
            if ky == 0:
                ok_y[0] = False
            if ky == 2:
                ok_y[H - 1] = False
            ok_x = np.ones(W, bool)
            if kx == 0:
                ok_x[0] = False
            if kx == 2:
                ok_x[W - 1] = False
            mk[25 + ky * 3 + kx] = ok_y[:, None] & ok_x[None, :]
    interior = (by >= 1) & (by <= NB - 2) & (bx >= 1) & (bx <= NB - 2)
    mk[34] = interior
    mk30 = np.zeros((KM, H, WP), np.float32)
    mk30[:, :, :W] = mk
    mk = mk30.reshape(KM, H * WP)

    # bias lhsT rows [10, 256]: rate2-folded tap biases + att bias
    bl = np.empty((10, C), np.float32)
    for s in range(9):
        bl[s] = (r2 * bA[s]).astype(np.float32)
    bl[9] = (r1 * bsrc[2]).astype(np.float32)

    return wa, w3p, mk, bl


def kernel(x, w1, b1, w2, b2, w3, b3, fc_w, dep_w, rel_height, rel_width,
           rate1, rate2):
    global _graph_cache, last_exec_time_ns
    if _graph_cache is None:
        _graph_cache = _build_graph()
    nc = _graph_cache

    wa, w3p, mk, bl = _host_precompute(
        w1, b1, w2, b2, w3, b3, fc_w, dep_w, rate1, rate2)

    import ml_dtypes
    np_cdt = ml_dtypes.bfloat16 if CONV_DTYPE == "bf16" else np.float32
    wa = wa.astype(np_cdt)
    w3p = w3p.astype(ml_dtypes.bfloat16)
    mk = mk.astype(ml_dtypes.bfloat16)
    bl = bl.astype(ml_dtypes.bfloat16)
    xpad = np.pad(np.ascontiguousarray(x, dtype=np.float32),
                  ((0, 0), (0, 0), (1, 1), (1, 1)))
    xpad = xpad.reshape(B, C, HP * WP).astype(np_cdt)

    in_maps = []
    for i in range(NCORES):
        in_maps.append({
            "xp": np.ascontiguousarray(xpad[i * BLOC:(i + 1) * BLOC]),
            "wa": wa, "w3p": w3p, "mk": mk, "bl": bl,
        })

    kw = {}
    if TRACE:
        import tempfile
        kw["tmpdir"] = tempfile.mkdtemp(prefix="ktrace_", dir="/tmp")
        globals()["last_trace_dir"] = kw["tmpdir"]
    res = run_bass_kernel_spmd(nc, in_maps, core_ids=list(range(NCORES)),
                               trace=TRACE, **kw)
    last_exec_time_ns = res.exec_time_ns
    out = np.concatenate([res.results[i]["out"] for i in range(NCORES)], axis=0)
    return out.reshape(B, C, H, W)
